# revision 23
# baseline (speedup 1.0000x reference)
"""BlockWiseEmbedding kernel for 8 Trainium2 NeuronCores.

Strategy (data-parallel tokens, replicated tables):
  - Host: route each token to its block via block_assignment/local_assignment
    (pure index bookkeeping on small int arrays), dedup rows per block, and
    deal each block's unique rows evenly across the 8 cores.
  - Device (identical SPMD program on all 8 cores): for each block b,
    dma_gather the routed embedding rows from the block table in HBM into
    SBUF [128 tokens x s_b] (f32), cast to bf16 on DVE, transpose 128x128
    tiles through the PE into [s_b x 128 tokens], matmul (bf16) against the
    resident bf16 transformer weights [s_b x 512] accumulating in PSUM f32,
    and DMA the [tokens x 512] result (bf16) to the per-core output buffer.
  - Host: scatter per-core outputs back to original token order (f32).

v2 changes over the 54.6us baseline (trace-driven):
  - idx DMA issued FIRST on sync; weights moved to the scalar (ACT) HWDGE
    queue.  In the baseline the 30KB idx buffer completed only at ~19us
    (queued behind 2MB of weights), and the first Q7 gather desc-gen -- the
    pacing resource, ~700ns + 8ns/idx serial per call -- started then.
  - whole matmul path in bf16: weights are pre-cast on host (halves weight
    DMA), gathered rows cast f32->bf16 on DVE, transposes + matmuls run in
    bf16 (fp32 ran at ~2cyc/col; bf16 streams 1 col/cycle), output stored
    bf16 (host casts back; PSUM accumulation stays f32).
"""

import os
import sys

import numpy as np

for _p in ("/opt/trn_rl_repo", "/root/.axon_site/_ro/trn_rl_repo"):
    if os.path.isdir(_p) and _p not in sys.path:
        sys.path.append(_p)

N_CORES = 8
OUT_DIM = 512
N_BLOCKS = 4

TRACE = False
# dummy PE transposes issued while waiting for the GPSIMD library load +
# first gather, keeping the PE pipeline ramped to max pstate
PE_WARMUP = 140
# the first block's gather is split so its first chunk reaches the PE a
# desc-gen earlier; later blocks unsplit (per-call fixed cost ~0.7us)
GATHER_SPLIT = 2

LAST_EXEC_NS = None
LAST_RESULTS = None

_CACHE = {}


def _cdiv(a, b):
    return -(-a // b)


def _build_program(sizes, table_rows, nb16, out_dim):
    import concourse.mybir as mybir
    from concourse import bacc, tile
    from concourse._compat import get_trn_type
    from concourse.library_config import mlp

    f32 = mybir.dt.float32
    bf16 = mybir.dt.bfloat16
    i16 = mybir.dt.int16
    nB = len(sizes)
    offs = [0]
    for n in nb16:
        offs.append(offs[-1] + n)
    tot = offs[-1]
    totcols = tot // 16

    # process big blocks first: equal gather desc-gen cost per block, but the
    # big block carries the most PE work -- start it earliest
    border = sorted(range(nB), key=lambda b: -sizes[b])

    nc = bacc.Bacc(get_trn_type() or "TRN2", target_bir_lowering=False)
    tabs = [
        nc.dram_tensor(f"block{b}", [table_rows[b], sizes[b]], f32, kind="ExternalInput")
        for b in range(nB)
    ]
    trs = [
        nc.dram_tensor(f"trans{b}", [sizes[b], out_dim], bf16, kind="ExternalInput")
        for b in range(nB)
    ]
    idx = nc.dram_tensor("idx", [128, totcols], i16, kind="ExternalInput")
    identh = nc.dram_tensor("ident", [128, 128], bf16, kind="ExternalInput")
    out = nc.dram_tensor("out", [tot, out_dim], bf16, kind="ExternalOutput")

    nc.gpsimd.load_library(mlp)

    # engine-balance for DVE/ACT copies: DVE is faster per element, weight
    # the split so both finish together
    load = {"v": 0.0, "s": 0.0}

    def copy_bal(dst, src, elems):
        if load["v"] <= load["s"]:
            nc.vector.tensor_copy(dst, src)
            load["v"] += elems
        else:
            nc.scalar.copy(dst, src)
            load["s"] += elems * 1.7

    with tile.TileContext(nc) as tc:
        with (
            tc.tile_pool(name="const", bufs=1) as cpool,
            tc.tile_pool(name="gath", bufs=1) as gpool,
            tc.tile_pool(name="et", bufs=8) as epool,
            tc.tile_pool(name="ot", bufs=4) as opool,
            tc.tile_pool(name="pt", bufs=3, space="PSUM") as ptpool,
            tc.tile_pool(name="po", bufs=4, space="PSUM") as popool,
            tc.tile_pool(name="warm", bufs=1, space="PSUM") as wpool,
        ):
            # idx first: the first gather's desc-gen (the pacing resource)
            # waits on it; everything else can land later
            idx_sb = cpool.tile([128, totcols], i16)
            nc.sync.dma_start(idx_sb[:], idx[:, :])
            ident = cpool.tile([128, 128], bf16)
            nc.sync.dma_start(ident[:], identh[:, :])
            # Weight loads are WAW-gated behind the idx DMA via dummy DVE
            # writes into each weight tile reading idx_sb.  The GPSIMD
            # library IRAM load is itself an SDMA transfer that effectively
            # drains after all other in-flight traffic, and the first gather
            # desc-gen (the pacing resource) waits on it -- keeping the bulk
            # weight DMA out of the earliest window limits that delay, while
            # weights still land well before the first matmul.  (The dummy
            # must NOT read a tile that later warmup transposes write: that
            # creates a WAR cycle the scheduler breaks by deferring the rest
            # of the warmup, leaving the PE cold.)
            idx_probe = idx_sb[0:1, 0:16].bitcast(bf16)
            tr_sb = [None] * nB
            for b in border:
                s = sizes[b]
                p = min(128, s)
                nk = _cdiv(s, 128)
                t = cpool.tile([p, nk, out_dim], bf16, tag=f"tr{b}")
                nc.vector.tensor_copy(t[0:1, 0, 0:16], idx_probe)
                tr_sb[b] = t
            for b in border:
                s = sizes[b]
                p = min(128, s)
                nc.sync.dma_start(
                    tr_sb[b][:],
                    trs[b][:, :].rearrange("(k p) n -> p k n", p=p),
                )

            # keep the PE hot while the GPSIMD library loads and the first
            # gather's descriptors are generated: dummy ident transposes,
            # serialized by WAW on one PSUM tile
            warm = wpool.tile([128, 128], bf16, tag="warm")
            for i in range(PE_WARMUP):
                nc.tensor.transpose(warm[:], ident[:], ident[:])

            g_sb = [None] * nB
            gb_sb = [None] * nB
            gather_parts = {}  # b -> list of (chunk_lo, chunk_hi) per part
            for b in border:
                if nb16[b] == 0:
                    continue
                s = sizes[b]
                C = _cdiv(nb16[b], 128)
                g = gpool.tile([128, C, s], f32, tag=f"g{b}")
                if nb16[b] % 128 != 0:
                    # zero the partial last chunk so token slots the gather
                    # won't write stay finite downstream
                    nc.vector.memset(g[:, C - 1, :], 0.0)
                g_sb[b] = g
                gb = gpool.tile([128, C, s], bf16, tag=f"gb{b}", name=f"gb{b}")
                gb_sb[b] = gb
                # first block: two equal parts so the PE is fed steadily
                # (a tiny first part leaves a >3.4us data gap before part 2
                # lands -- the PE goes idle and HAM rethrottles the clock);
                # later blocks unsplit (per-call fixed cost ~0.7us)
                if b == border[0] and GATHER_SPLIT > 1 and C > 1:
                    h = C // 2
                    gather_parts[b] = [(0, h), (h, C)]
                else:
                    gather_parts[b] = [(0, C)]

            def emit_gather(b, lo, hi):
                s = sizes[b]
                n_idx = min(nb16[b], hi * 128) - lo * 128
                nc.gpsimd.dma_gather(
                    g_sb[b][:, lo:hi, :],
                    tabs[b][:, :],
                    idx_sb[
                        :,
                        offs[b] // 16 + lo * 8 : offs[b] // 16 + lo * 8 + n_idx // 16,
                    ],
                    n_idx,
                    n_idx,
                    s,
                )

            # sequential per block, matching PE consumption order -- the split
            # just gets the first chunk's data to the PE one part sooner
            for b in border:
                if nb16[b]:
                    for lo, hi in gather_parts[b]:
                        emit_gather(b, lo, hi)



            for b in border:
                if nb16[b] == 0:
                    continue
                s = sizes[b]
                nk = _cdiv(s, 128)
                C = _cdiv(nb16[b], 128)
                for lo, hi in gather_parts[b]:
                    for m in range(lo, hi):
                        # cast this chunk's gathered rows to bf16 on ACT (not
                        # DVE: GPSIMD shares an SBUF port with DVE, and a
                        # bulk cast waiting on a gather sem at the DVE queue
                        # head head-of-line blocks the PSUM copies behind it)
                        nc.scalar.copy(
                            gb_sb[b][:, m, :], g_sb[b][:, m, :]
                        )
                        load["s"] += 128 * s * 1.7
                        rows = min(128, nb16[b] - m * 128)
                        # all k-slices of this chunk transpose into one wide
                        # PSUM tile, then one copy to SBUF: fewer instructions
                        # and semaphores than per-slice tiles
                        prow = min(128, s)
                        pt = ptpool.tile([128, nk * 128], bf16, tag="pt")
                        for k in range(nk):
                            ks = min(128, s - k * 128)
                            nc.tensor.transpose(
                                pt[:ks, k * 128 : k * 128 + 128],
                                gb_sb[b][:, m, k * 128 : k * 128 + ks],
                                ident[:],
                            )
                        et = epool.tile([128, nk * 128], bf16, tag="et")
                        copy_bal(
                            et[:prow, :], pt[:prow, :], prow * nk * 128 * 0.5
                        )
                        po = popool.tile([128, out_dim], f32, tag="po")
                        for k in range(nk):
                            ks = min(128, s - k * 128)
                            nc.tensor.matmul(
                                po[:, :],
                                et[:ks, k * 128 : k * 128 + 128],
                                tr_sb[b][:ks, k, :],
                                start=(k == 0),
                                stop=(k == nk - 1),
                            )
                        ot = opool.tile([128, out_dim], bf16, tag="ot")
                        copy_bal(ot[:rows, :], po[:rows, :], rows * out_dim)
                        nc.sync.dma_start(
                            out[offs[b] + m * 128 : offs[b] + m * 128 + rows, :],
                            ot[:rows, :],
                        )

    nc.compile()
    return nc, offs, tot


def _route(src, block_assignment, local_assignment, table_rows):
    """Host-side token routing with row dedup. Each block's referenced table
    rows are deduplicated (np.unique, so per-core gather indices are sorted
    ascending -> better HBM locality) and dealt evenly across cores. Returns
    per-core index buffers plus bookkeeping to reassemble outputs."""
    src_f = np.asarray(src).reshape(-1)
    ba = np.asarray(block_assignment)[src_f]
    la = np.asarray(local_assignment)[src_f]

    nb = [0] * N_BLOCKS
    nb16 = [0] * N_BLOCKS
    # per block: (token_ids, row_position_of_each_token, urows)
    binfo = []
    for b in range(N_BLOCKS):
        toks = np.where(ba == b)[0]
        rows = np.clip(la[toks], 0, table_rows[b] - 1)
        urows, inv = np.unique(rows, return_inverse=True)
        binfo.append((toks, inv, urows))
        nb[b] = int(_cdiv(urows.size, N_CORES))
        nb16[b] = _cdiv(nb[b], 16) * 16

    offs = [0]
    for n in nb16:
        offs.append(offs[-1] + n)
    tot = offs[-1]
    totcols = tot // 16

    idx_bufs = np.zeros((N_CORES, 128, totcols), dtype=np.int16)
    for b in range(N_BLOCKS):
        toks, inv, urows = binfo[b]
        if urows.size == 0:
            continue
        for c in range(N_CORES):
            lo = c * nb[b]
            hi = min(urows.size, lo + nb[b])
            if hi <= lo:
                continue
            pad = np.zeros((nb16[b],), dtype=np.int16)
            pad[: hi - lo] = urows[lo:hi].astype(np.int16)
            # index j lives at [j % 16, j // 16], segment starts at column
            # offs[b] // 16; the 16-partition block is replicated to all 128
            # partitions (each Q7 core pair reads its own copy)
            wrapped = pad.reshape(-1, 16).T  # [16, nb16/16]
            idx_bufs[c, :, offs[b] // 16 : offs[b] // 16 + nb16[b] // 16] = np.tile(
                wrapped, (8, 1)
            )
    return idx_bufs, binfo, tuple(nb), tuple(nb16), offs, tot


def _bf16(a):
    import ml_dtypes

    return np.asarray(a).astype(ml_dtypes.bfloat16)


def _make_in_map(idx_buf, blocks, trans_bf16, ident_bf16):
    m = {"idx": idx_buf, "ident": ident_bf16}
    for b in range(N_BLOCKS):
        m[f"block{b}"] = blocks[b]
        m[f"trans{b}"] = trans_bf16[b]
    return m


def _ensure_ntff_hook():
    """Register the axon NTFF profiling hook if the image's antenv lacks it."""
    try:
        from antenv.axon_hooks import get_axon_ntff_profile_hook  # noqa: F401

        return
    except ImportError:
        pass
    import types

    mod = types.ModuleType("antenv.axon_hooks")
    holder = {"h": None}
    mod.set_axon_ntff_profile_hook = lambda h: holder.__setitem__("h", h)
    mod.get_axon_ntff_profile_hook = lambda: holder["h"]
    sys.modules["antenv.axon_hooks"] = mod
    try:
        if "/root/.axon_site" not in sys.path:
            sys.path.append("/root/.axon_site")
        from trn_agent_boot.trn_boot import _ntff_profile_via_ctypes

        so = "/opt/axon/libaxon_pjrt.so"
        if os.path.exists(so):
            h = _ntff_profile_via_ctypes(so)
            if h is not None:
                mod.set_axon_ntff_profile_hook(h)
    except Exception:
        pass


def kernel(
    src,
    block_assignment,
    local_assignment,
    block0,
    block1,
    block2,
    block3,
    trans0,
    trans1,
    trans2,
    trans3,
):
    global LAST_EXEC_NS, LAST_RESULTS
    from concourse.bass_utils import run_bass_kernel_spmd

    blocks = [np.ascontiguousarray(np.asarray(x), dtype=np.float32)
              for x in (block0, block1, block2, block3)]
    trans = [_bf16(x) for x in (trans0, trans1, trans2, trans3)]
    sizes = [b.shape[1] for b in blocks]
    table_rows = [b.shape[0] for b in blocks]
    src = np.asarray(src)

    idx_bufs, binfo, nb, nb16, offs, tot = _route(
        src, block_assignment, local_assignment, table_rows
    )

    key = (tuple(sizes), tuple(table_rows), nb16)
    if key not in _CACHE:
        _CACHE[key] = _build_program(sizes, table_rows, list(nb16), OUT_DIM)
    nc, _, _ = _CACHE[key]

    ident = _bf16(np.eye(128, dtype=np.float32))
    in_maps = [
        _make_in_map(idx_bufs[c], blocks, trans, ident) for c in range(N_CORES)
    ]

    if TRACE:
        _ensure_ntff_hook()
        import concourse.bass_utils as _bu

        if not getattr(_bu, "_upload_patched", False):
            _bu.upload_artifacts = lambda d: "local://" + d
            _bu._upload_patched = True
        try:
            res = run_bass_kernel_spmd(
                nc, in_maps, core_ids=list(range(N_CORES)), trace=True
            )
        except Exception:
            res = run_bass_kernel_spmd(
                nc, in_maps, core_ids=list(range(N_CORES)), trace=False
            )
    else:
        res = run_bass_kernel_spmd(
            nc, in_maps, core_ids=list(range(N_CORES)), trace=False
        )
    LAST_EXEC_NS = res.exec_time_ns
    LAST_RESULTS = res

    T = src.size
    out_flat = np.zeros((T, OUT_DIM), dtype=np.float32)
    all_out = np.stack(
        [np.asarray(res.results[c]["out"]).astype(np.float32) for c in range(N_CORES)]
    )
    for b in range(N_BLOCKS):
        toks, inv, urows = binfo[b]
        if urows.size == 0:
            continue
        core = inv // nb[b]
        pos = inv % nb[b]
        out_flat[toks] = all_out[core, offs[b] + pos]
    return out_flat.reshape(src.shape + (OUT_DIM,))


# revision 25
# speedup vs baseline: 1.0330x; 1.0330x over previous
"""BlockWiseEmbedding kernel for 8 Trainium2 NeuronCores.

Strategy (data-parallel tokens, replicated tables):
  - Host: route each token to its block via block_assignment/local_assignment
    (pure index bookkeeping on small int arrays), dedup rows per block, and
    deal each block's unique rows evenly across the 8 cores.
  - Device (identical SPMD program on all 8 cores): for each block b,
    dma_gather the routed embedding rows from the block table in HBM into
    SBUF [128 tokens x s_b] (f32), cast to bf16 on DVE, transpose 128x128
    tiles through the PE into [s_b x 128 tokens], matmul (bf16) against the
    resident bf16 transformer weights [s_b x 512] accumulating in PSUM f32,
    and DMA the [tokens x 512] result (bf16) to the per-core output buffer.
  - Host: scatter per-core outputs back to original token order (f32).

v2 changes over the 54.6us baseline (trace-driven):
  - idx DMA issued FIRST on sync; weights moved to the scalar (ACT) HWDGE
    queue.  In the baseline the 30KB idx buffer completed only at ~19us
    (queued behind 2MB of weights), and the first Q7 gather desc-gen -- the
    pacing resource, ~700ns + 8ns/idx serial per call -- started then.
  - whole matmul path in bf16: weights are pre-cast on host (halves weight
    DMA), gathered rows cast f32->bf16 on DVE, transposes + matmuls run in
    bf16 (fp32 ran at ~2cyc/col; bf16 streams 1 col/cycle), output stored
    bf16 (host casts back; PSUM accumulation stays f32).
"""

import os
import sys

import numpy as np

for _p in ("/opt/trn_rl_repo", "/root/.axon_site/_ro/trn_rl_repo"):
    if os.path.isdir(_p) and _p not in sys.path:
        sys.path.append(_p)

N_CORES = 8
OUT_DIM = 512
N_BLOCKS = 4

TRACE = False
# dummy PE transposes issued while waiting for the GPSIMD library load +
# first gather, keeping the PE pipeline ramped to max pstate
PE_WARMUP = 85
# the first block's gather is split so its first chunk reaches the PE a
# desc-gen earlier; later blocks unsplit (per-call fixed cost ~0.7us)
GATHER_SPLIT = 2

LAST_EXEC_NS = None
LAST_RESULTS = None

_CACHE = {}


def _cdiv(a, b):
    return -(-a // b)


def _build_program(sizes, table_rows, nb16, out_dim):
    import concourse.mybir as mybir
    from concourse import bacc, tile
    from concourse._compat import get_trn_type
    from concourse.library_config import mlp

    f32 = mybir.dt.float32
    bf16 = mybir.dt.bfloat16
    i16 = mybir.dt.int16
    nB = len(sizes)
    offs = [0]
    for n in nb16:
        offs.append(offs[-1] + n)
    tot = offs[-1]
    totcols = tot // 16

    # process big blocks first: equal gather desc-gen cost per block, but the
    # big block carries the most PE work -- start it earliest
    border = sorted(range(nB), key=lambda b: -sizes[b])

    nc = bacc.Bacc(get_trn_type() or "TRN2", target_bir_lowering=False)
    tabs = [
        nc.dram_tensor(f"block{b}", [table_rows[b], sizes[b]], f32, kind="ExternalInput")
        for b in range(nB)
    ]
    trs = [
        nc.dram_tensor(f"trans{b}", [sizes[b], out_dim], bf16, kind="ExternalInput")
        for b in range(nB)
    ]
    idx = nc.dram_tensor("idx", [128, totcols], i16, kind="ExternalInput")
    identh = nc.dram_tensor("ident", [128, 128], bf16, kind="ExternalInput")
    out = nc.dram_tensor("out", [tot, out_dim], bf16, kind="ExternalOutput")

    nc.gpsimd.load_library(mlp)

    # engine-balance for DVE/ACT copies: DVE is faster per element, weight
    # the split so both finish together
    load = {"v": 0.0, "s": 0.0}

    def copy_bal(dst, src, elems):
        if load["v"] <= load["s"]:
            nc.vector.tensor_copy(dst, src)
            load["v"] += elems
        else:
            nc.scalar.copy(dst, src)
            load["s"] += elems * 1.7

    with tile.TileContext(nc) as tc:
        with (
            tc.tile_pool(name="const", bufs=1) as cpool,
            tc.tile_pool(name="gath", bufs=1) as gpool,
            tc.tile_pool(name="et", bufs=8) as epool,
            tc.tile_pool(name="ot", bufs=4) as opool,
            tc.tile_pool(name="pt", bufs=3, space="PSUM") as ptpool,
            tc.tile_pool(name="po", bufs=4, space="PSUM") as popool,
            tc.tile_pool(name="warm", bufs=1, space="PSUM") as wpool,
        ):
            # idx first: the first gather's desc-gen (the pacing resource)
            # waits on it; everything else can land later
            idx_sb = cpool.tile([128, totcols], i16)
            nc.sync.dma_start(idx_sb[:], idx[:, :])
            ident = cpool.tile([128, 128], bf16)
            nc.sync.dma_start(ident[:], identh[:, :])
            # Weight loads are WAW-gated behind the idx DMA via dummy DVE
            # writes into each weight tile reading idx_sb.  The GPSIMD
            # library IRAM load is itself an SDMA transfer that effectively
            # drains after all other in-flight traffic, and the first gather
            # desc-gen (the pacing resource) waits on it -- keeping the bulk
            # weight DMA out of the earliest window limits that delay, while
            # weights still land well before the first matmul.  (The dummy
            # must NOT read a tile that later warmup transposes write: that
            # creates a WAR cycle the scheduler breaks by deferring the rest
            # of the warmup, leaving the PE cold.)
            idx_probe = idx_sb[0:1, 0:16].bitcast(bf16)
            tr_sb = [None] * nB
            for b in border:
                s = sizes[b]
                p = min(128, s)
                nk = _cdiv(s, 128)
                t = cpool.tile([p, nk, out_dim], bf16, tag=f"tr{b}")
                nc.vector.tensor_copy(t[0:1, 0, 0:16], idx_probe)
                tr_sb[b] = t
            for b in border:
                s = sizes[b]
                p = min(128, s)
                nc.sync.dma_start(
                    tr_sb[b][:],
                    trs[b][:, :].rearrange("(k p) n -> p k n", p=p),
                )

            # keep the PE hot while the GPSIMD library loads and the first
            # gather's descriptors are generated: dummy ident transposes,
            # serialized by WAW on one PSUM tile
            warm = wpool.tile([128, 128], bf16, tag="warm")
            for i in range(PE_WARMUP):
                nc.tensor.transpose(warm[:], ident[:], ident[:])

            g_sb = [None] * nB
            gb_sb = [None] * nB
            gather_parts = {}  # b -> list of (chunk_lo, chunk_hi) per part
            for b in border:
                if nb16[b] == 0:
                    continue
                s = sizes[b]
                C = _cdiv(nb16[b], 128)
                g = gpool.tile([128, C, s], f32, tag=f"g{b}")
                if nb16[b] % 128 != 0:
                    # zero the partial last chunk so token slots the gather
                    # won't write stay finite downstream
                    nc.vector.memset(g[:, C - 1, :], 0.0)
                g_sb[b] = g
                gb = gpool.tile([128, C, s], bf16, tag=f"gb{b}", name=f"gb{b}")
                gb_sb[b] = gb
                # first block: one small leading part (1 chunk) to get the PE
                # going, remainder in one instruction; later blocks unsplit
                # (per-call fixed cost ~0.7us)
                if b == border[0] and GATHER_SPLIT > 1 and C > 1:
                    gather_parts[b] = [(0, 1), (1, C)]
                else:
                    gather_parts[b] = [(0, C)]

            def emit_gather(b, lo, hi):
                s = sizes[b]
                n_idx = min(nb16[b], hi * 128) - lo * 128
                nc.gpsimd.dma_gather(
                    g_sb[b][:, lo:hi, :],
                    tabs[b][:, :],
                    idx_sb[
                        :,
                        offs[b] // 16 + lo * 8 : offs[b] // 16 + lo * 8 + n_idx // 16,
                    ],
                    n_idx,
                    n_idx,
                    s,
                )

            # sequential per block, matching PE consumption order -- the split
            # just gets the first chunk's data to the PE one part sooner
            for b in border:
                if nb16[b]:
                    for lo, hi in gather_parts[b]:
                        emit_gather(b, lo, hi)



            for b in border:
                if nb16[b] == 0:
                    continue
                s = sizes[b]
                nk = _cdiv(s, 128)
                C = _cdiv(nb16[b], 128)
                for lo, hi in gather_parts[b]:
                    for m in range(lo, hi):
                        # cast this chunk's gathered rows to bf16 on ACT (not
                        # DVE: GPSIMD shares an SBUF port with DVE, and a
                        # bulk cast waiting on a gather sem at the DVE queue
                        # head head-of-line blocks the PSUM copies behind it)
                        nc.scalar.copy(
                            gb_sb[b][:, m, :], g_sb[b][:, m, :]
                        )
                        load["s"] += 128 * s * 1.7
                        rows = min(128, nb16[b] - m * 128)
                        # all k-slices of this chunk transpose into one wide
                        # PSUM tile, then one copy to SBUF: fewer instructions
                        # and semaphores than per-slice tiles
                        prow = min(128, s)
                        pt = ptpool.tile([128, nk * 128], bf16, tag="pt")
                        for k in range(nk):
                            ks = min(128, s - k * 128)
                            nc.tensor.transpose(
                                pt[:ks, k * 128 : k * 128 + 128],
                                gb_sb[b][:, m, k * 128 : k * 128 + ks],
                                ident[:],
                            )
                        et = epool.tile([128, nk * 128], bf16, tag="et")
                        copy_bal(
                            et[:prow, :], pt[:prow, :], prow * nk * 128 * 0.5
                        )
                        po = popool.tile([128, out_dim], f32, tag="po")
                        for k in range(nk):
                            ks = min(128, s - k * 128)
                            nc.tensor.matmul(
                                po[:, :],
                                et[:ks, k * 128 : k * 128 + 128],
                                tr_sb[b][:ks, k, :],
                                start=(k == 0),
                                stop=(k == nk - 1),
                            )
                        ot = opool.tile([128, out_dim], bf16, tag="ot")
                        copy_bal(ot[:rows, :], po[:rows, :], rows * out_dim)
                        nc.sync.dma_start(
                            out[offs[b] + m * 128 : offs[b] + m * 128 + rows, :],
                            ot[:rows, :],
                        )

    nc.compile()
    return nc, offs, tot


def _route(src, block_assignment, local_assignment, table_rows):
    """Host-side token routing with row dedup. Each block's referenced table
    rows are deduplicated (np.unique, so per-core gather indices are sorted
    ascending -> better HBM locality) and dealt evenly across cores. Returns
    per-core index buffers plus bookkeeping to reassemble outputs."""
    src_f = np.asarray(src).reshape(-1)
    ba = np.asarray(block_assignment)[src_f]
    la = np.asarray(local_assignment)[src_f]

    nb = [0] * N_BLOCKS
    nb16 = [0] * N_BLOCKS
    # per block: (token_ids, row_position_of_each_token, urows)
    binfo = []
    for b in range(N_BLOCKS):
        toks = np.where(ba == b)[0]
        rows = np.clip(la[toks], 0, table_rows[b] - 1)
        urows, inv = np.unique(rows, return_inverse=True)
        binfo.append((toks, inv, urows))
        nb[b] = int(_cdiv(urows.size, N_CORES))
        nb16[b] = _cdiv(nb[b], 16) * 16

    offs = [0]
    for n in nb16:
        offs.append(offs[-1] + n)
    tot = offs[-1]
    totcols = tot // 16

    idx_bufs = np.zeros((N_CORES, 128, totcols), dtype=np.int16)
    for b in range(N_BLOCKS):
        toks, inv, urows = binfo[b]
        if urows.size == 0:
            continue
        for c in range(N_CORES):
            lo = c * nb[b]
            hi = min(urows.size, lo + nb[b])
            if hi <= lo:
                continue
            pad = np.zeros((nb16[b],), dtype=np.int16)
            pad[: hi - lo] = urows[lo:hi].astype(np.int16)
            # index j lives at [j % 16, j // 16], segment starts at column
            # offs[b] // 16; the 16-partition block is replicated to all 128
            # partitions (each Q7 core pair reads its own copy)
            wrapped = pad.reshape(-1, 16).T  # [16, nb16/16]
            idx_bufs[c, :, offs[b] // 16 : offs[b] // 16 + nb16[b] // 16] = np.tile(
                wrapped, (8, 1)
            )
    return idx_bufs, binfo, tuple(nb), tuple(nb16), offs, tot


def _bf16(a):
    import ml_dtypes

    return np.asarray(a).astype(ml_dtypes.bfloat16)


def _make_in_map(idx_buf, blocks, trans_bf16, ident_bf16):
    m = {"idx": idx_buf, "ident": ident_bf16}
    for b in range(N_BLOCKS):
        m[f"block{b}"] = blocks[b]
        m[f"trans{b}"] = trans_bf16[b]
    return m


def _ensure_ntff_hook():
    """Register the axon NTFF profiling hook if the image's antenv lacks it."""
    try:
        from antenv.axon_hooks import get_axon_ntff_profile_hook  # noqa: F401

        return
    except ImportError:
        pass
    import types

    mod = types.ModuleType("antenv.axon_hooks")
    holder = {"h": None}
    mod.set_axon_ntff_profile_hook = lambda h: holder.__setitem__("h", h)
    mod.get_axon_ntff_profile_hook = lambda: holder["h"]
    sys.modules["antenv.axon_hooks"] = mod
    try:
        if "/root/.axon_site" not in sys.path:
            sys.path.append("/root/.axon_site")
        from trn_agent_boot.trn_boot import _ntff_profile_via_ctypes

        so = "/opt/axon/libaxon_pjrt.so"
        if os.path.exists(so):
            h = _ntff_profile_via_ctypes(so)
            if h is not None:
                mod.set_axon_ntff_profile_hook(h)
    except Exception:
        pass


def kernel(
    src,
    block_assignment,
    local_assignment,
    block0,
    block1,
    block2,
    block3,
    trans0,
    trans1,
    trans2,
    trans3,
):
    global LAST_EXEC_NS, LAST_RESULTS
    from concourse.bass_utils import run_bass_kernel_spmd

    blocks = [np.ascontiguousarray(np.asarray(x), dtype=np.float32)
              for x in (block0, block1, block2, block3)]
    trans = [_bf16(x) for x in (trans0, trans1, trans2, trans3)]
    sizes = [b.shape[1] for b in blocks]
    table_rows = [b.shape[0] for b in blocks]
    src = np.asarray(src)

    idx_bufs, binfo, nb, nb16, offs, tot = _route(
        src, block_assignment, local_assignment, table_rows
    )

    key = (tuple(sizes), tuple(table_rows), nb16)
    if key not in _CACHE:
        _CACHE[key] = _build_program(sizes, table_rows, list(nb16), OUT_DIM)
    nc, _, _ = _CACHE[key]

    ident = _bf16(np.eye(128, dtype=np.float32))
    in_maps = [
        _make_in_map(idx_bufs[c], blocks, trans, ident) for c in range(N_CORES)
    ]

    if TRACE:
        _ensure_ntff_hook()
        import concourse.bass_utils as _bu

        if not getattr(_bu, "_upload_patched", False):
            _bu.upload_artifacts = lambda d: "local://" + d
            _bu._upload_patched = True
        try:
            res = run_bass_kernel_spmd(
                nc, in_maps, core_ids=list(range(N_CORES)), trace=True
            )
        except Exception:
            res = run_bass_kernel_spmd(
                nc, in_maps, core_ids=list(range(N_CORES)), trace=False
            )
    else:
        res = run_bass_kernel_spmd(
            nc, in_maps, core_ids=list(range(N_CORES)), trace=False
        )
    LAST_EXEC_NS = res.exec_time_ns
    LAST_RESULTS = res

    T = src.size
    out_flat = np.zeros((T, OUT_DIM), dtype=np.float32)
    all_out = np.stack(
        [np.asarray(res.results[c]["out"]).astype(np.float32) for c in range(N_CORES)]
    )
    for b in range(N_BLOCKS):
        toks, inv, urows = binfo[b]
        if urows.size == 0:
            continue
        core = inv // nb[b]
        pos = inv % nb[b]
        out_flat[toks] = all_out[core, offs[b] + pos]
    return out_flat.reshape(src.shape + (OUT_DIM,))
